# revision 7
# baseline (speedup 1.0000x reference)
"""Trainium2 Bass kernel for nn_ConvSSMBlockFourier.

The module computes, per (batch, channel) plane:
    y = crop( irfft2( rfft2(pad(x)) * Bf * sum_{t<8} Af^t ) )
which is a per-channel linear 2D convolution of x (128x128) with an
effective real kernel E_c of support 49x49 (E_c = B_c conv sum_t A_dec_c^{conv t}).

Strategy:
  - Host: compute E_c exactly (64x64 FFT grid, fp64), then its 176x176
    half-spectrum Ehat_c (89 x 176 complex).  176 >= 128+49-1 so the
    circular conv equals the linear conv exactly.
  - Device (per core, 32 channels x 4 batch planes = 128 planes):
    DFT-by-matmul pipeline on TensorE (f32r full-rate matmuls):
      A: psumA = X^T @ FA                 (rfft along i, half spectrum)
      B: psumV = Ur^T.T @ FB1 + Ui^T.T @ FB2   ([Vr | Vi], full fft along j)
      .: Z = V * Ehat  (complex pointwise, VectorE, batched over 4 planes)
      C: psumG = Zr^T.T @ CC1 + Zi^T.T @ CC2   ([Gr^T | Gi^T], inverse-i + crop)
      D: psumY = WD^T @ G                  (inverse-j + crop, real output, 2 planes/MM)
    ScalarE does the PSUM->SBUF copies between matmul stages.
  - Sharding: channels across the 8 cores (32 each); all cores independent.
"""

import numpy as np

from concourse import bacc, bass, mybir, tile
from concourse import bass_utils

H = 128
W = 128
B = 4
C = 256
K = 7
T_STEPS = 8
DECAY = 0.3
S = 176              # FFT size both axes (>= 128 + 49 - 1)
BINS = S // 2 + 1    # 89 rfft bins along i
N_CORES = 8
CH_PER_CORE = C // N_CORES  # 32

f32 = mybir.dt.float32
f32r = mybir.dt.float32r

_COMPILED = {}


# ----------------------------------------------------------------------------
# host-side precompute
# ----------------------------------------------------------------------------

def _host_effective_kernel(A_kernel, B_kernel):
    """Ehat: (C, BINS, 2, S) float32 — [real | imag] half spectrum per channel."""
    dec = np.exp(-DECAY * np.arange(K))
    A_dec = A_kernel.astype(np.float64) * (dec[:, None] * dec[None, :])
    G = 64  # supports: A^{conv 7} -> 43, B conv that -> 49 <= 64
    Af = np.fft.fft2(A_dec, s=(G, G))
    Bf = np.fft.fft2(B_kernel.astype(np.float64), s=(G, G))
    geo = np.ones_like(Af)
    term = np.ones_like(Af)
    for _ in range(T_STEPS - 1):
        term = term * Af
        geo = geo + term
    E = np.real(np.fft.ifft2(Bf * geo))[:, :49, :49]          # (C, 49, 49)
    Ehat = np.fft.fft2(E, s=(S, S))[:, :BINS, :]              # (C, 89, 176) complex
    out = np.empty((C, BINS, 2, S), np.float32)
    out[:, :, 0, :] = Ehat.real
    out[:, :, 1, :] = Ehat.imag
    return out


def _host_consts():
    i = np.arange(H)
    p = np.arange(BINS)
    q = np.arange(S)
    j = np.arange(W)

    ang_a = 2 * np.pi * np.outer(i, p) / S
    FA = np.zeros((H, 256), np.float32)                        # padded to 256 for f32r
    FA[:, :BINS] = np.cos(ang_a)
    FA[:, BINS:2 * BINS] = -np.sin(ang_a)

    ang_b = 2 * np.pi * np.outer(j, q) / S
    cb, sb = np.cos(ang_b), np.sin(ang_b)
    FB1 = np.hstack([cb, -sb]).astype(np.float32)              # (128, 352)
    FB2 = np.hstack([sb, cb]).astype(np.float32)               # (128, 352)

    w = np.full(BINS, 2.0 / (S * S))
    w[0] *= 0.5
    w[BINS - 1] *= 0.5
    ang_c = 2 * np.pi * np.outer(p, i) / S
    Cr = (w[:, None] * np.cos(ang_c)).astype(np.float32)
    Ci = (w[:, None] * np.sin(ang_c)).astype(np.float32)
    CC1 = np.hstack([Cr, Ci]).astype(np.float32)               # (89, 256)
    CC2 = np.hstack([-Ci, Cr]).astype(np.float32)              # (89, 256)

    ang_d = 2 * np.pi * np.outer(q, j) / S
    WD = np.hstack([np.cos(ang_d), -np.sin(ang_d)]).astype(np.float32)  # (176, 256)
    WD1 = np.ascontiguousarray(WD[:128])                       # (128, 256)
    WD2 = np.ascontiguousarray(WD[128:])                       # (48, 256)
    return FA, FB1, FB2, CC1, CC2, WD1, WD2


# ----------------------------------------------------------------------------
# device kernel
# ----------------------------------------------------------------------------

def _build_core_kernel(ctx, nc, tc, ins, outs, n_ch):
    x_d, eh_d, fa_d, fb1_d, fb2_d, cc1_d, cc2_d, wd1_d, wd2_d = ins
    y_d = outs[0]

    cp = ctx.enter_context(tc.tile_pool(name="consts", bufs=1))
    xp = ctx.enter_context(tc.tile_pool(name="x", bufs=4))
    up = ctx.enter_context(tc.tile_pool(name="u", bufs=3))
    ep = ctx.enter_context(tc.tile_pool(name="eh", bufs=2))
    sp = ctx.enter_context(tc.tile_pool(name="scratch", bufs=8))
    zp = ctx.enter_context(tc.tile_pool(name="z", bufs=2))
    gp = ctx.enter_context(tc.tile_pool(name="g", bufs=2))
    yp = ctx.enter_context(tc.tile_pool(name="y", bufs=2))
    pa = ctx.enter_context(tc.tile_pool(name="psA", bufs=1, space="PSUM"))
    pv = ctx.enter_context(tc.tile_pool(name="psV", bufs=2, space="PSUM"))
    pg1 = ctx.enter_context(tc.tile_pool(name="psG1", bufs=1, space="PSUM"))
    pg2 = ctx.enter_context(tc.tile_pool(name="psG2", bufs=1, space="PSUM"))
    py = ctx.enter_context(tc.tile_pool(name="psY", bufs=1, space="PSUM"))

    fa = cp.tile([H, 256], f32r)
    nc.sync.dma_start(fa[:], fa_d[:])
    fb1 = cp.tile([W, 2 * S], f32r)
    nc.sync.dma_start(fb1[:], fb1_d[:])
    fb2 = cp.tile([W, 2 * S], f32r)
    nc.sync.dma_start(fb2[:], fb2_d[:])
    cc1 = cp.tile([BINS, 256], f32r)
    nc.sync.dma_start(cc1[:], cc1_d[:])
    cc2 = cp.tile([BINS, 256], f32r)
    nc.sync.dma_start(cc2[:], cc2_d[:])
    wd1 = cp.tile([128, 256], f32r)
    nc.sync.dma_start(wd1[:], wd1_d[:])
    wd2 = cp.tile([S - 128, 256], f32r)
    nc.sync.dma_start(wd2[:], wd2_d[:])

    for c in range(n_ch):
        eh = ep.tile([BINS, 2, S], f32)
        nc.sync.dma_start(eh[:], eh_d[c])

        for pair in range(B // 2):
            psa = pa.tile([H, 2, 256], f32)
            psv = pv.tile([BINS, 2, 512], f32)
            for k in range(2):
                b = 2 * pair + k
                xt = xp.tile([H, W], f32r)
                nc.sync.dma_start(xt[:], x_d[c, b])
                nc.tensor.matmul(psa[:, k, :], xt[:], fa[:], start=True, stop=True)
                ut = up.tile([W, 2 * BINS], f32r)
                nc.scalar.copy(ut[:], psa[:, k, 0:2 * BINS])
                nc.tensor.matmul(psv[:, k, 0:2 * S], ut[:, 0:BINS], fb1[:],
                                 start=True, stop=False)
                nc.tensor.matmul(psv[:, k, 0:2 * S], ut[:, BINS:2 * BINS], fb2[:],
                                 start=False, stop=True)

            # pointwise complex multiply, batched over the 2 planes of the pair
            vr = psv[:, 0:2, 0:S]
            vi = psv[:, 0:2, S:2 * S]
            er, _ = bass.broadcast_tensor_aps(eh[:, 0:1, :], vr)
            ei, _ = bass.broadcast_tensor_aps(eh[:, 1:2, :], vr)
            z = zp.tile([BINS, 2, 2 * S], f32r)
            t1 = sp.tile([BINS, 2, S], f32, tag="t")
            nc.vector.tensor_mul(t1[:], vr, er)
            t2 = sp.tile([BINS, 2, S], f32, tag="t")
            nc.vector.tensor_mul(t2[:], vi, ei)
            nc.vector.tensor_sub(z[:, 0:2, 0:S], t1[:], t2[:])
            t3 = sp.tile([BINS, 2, S], f32, tag="t")
            nc.vector.tensor_mul(t3[:], vr, ei)
            t4 = sp.tile([BINS, 2, S], f32, tag="t")
            nc.vector.tensor_mul(t4[:], vi, er)
            nc.vector.tensor_add(z[:, 0:2, S:2 * S], t3[:], t4[:])

            psgA = pg1.tile([128, 2, 256], f32)
            psgB = pg2.tile([S - 128, 2, 256], f32)
            for k in range(2):
                nc.tensor.matmul(psgA[:, k, :], z[:, k, 0:128], cc1[:],
                                 start=True, stop=False)
                nc.tensor.matmul(psgA[:, k, :], z[:, k, S:S + 128], cc2[:],
                                 start=False, stop=True)
                nc.tensor.matmul(psgB[:, k, :], z[:, k, 128:S], cc1[:],
                                 start=True, stop=False)
                nc.tensor.matmul(psgB[:, k, :], z[:, k, S + 128:2 * S], cc2[:],
                                 start=False, stop=True)
            g1 = gp.tile([128, 2, 256], f32r, tag="g1")
            nc.scalar.copy(g1[:], psgA[:])
            g2 = gp.tile([S - 128, 2, 256], f32r, tag="g2")
            nc.scalar.copy(g2[:], psgB[:])

            psy = py.tile([128, 2, 128], f32)
            nc.tensor.matmul(psy[:], wd1[:, 0:128], g1[:, 0:2, 0:128],
                             start=True, stop=False)
            nc.tensor.matmul(psy[:], wd1[:, 128:256], g1[:, 0:2, 128:256],
                             start=False, stop=False)
            nc.tensor.matmul(psy[:], wd2[:, 0:128], g2[:, 0:2, 0:128],
                             start=False, stop=False)
            nc.tensor.matmul(psy[:], wd2[:, 128:256], g2[:, 0:2, 128:256],
                             start=False, stop=True)
            yt = yp.tile([128, 2, 128], f32)
            nc.vector.tensor_copy(yt[:], psy[:])
            for k in range(2):
                nc.sync.dma_start(y_d[c, 2 * pair + k], yt[:, k, :])


def _build(n_ch):
    if n_ch in _COMPILED:
        return _COMPILED[n_ch]
    nc = bacc.Bacc("TRN2", target_bir_lowering=False, debug=False)
    ins = [
        nc.dram_tensor("x", [n_ch, B, H, W], f32r, kind="ExternalInput").ap(),
        nc.dram_tensor("eh", [n_ch, BINS, 2, S], f32, kind="ExternalInput").ap(),
        nc.dram_tensor("fa", [H, 256], f32r, kind="ExternalInput").ap(),
        nc.dram_tensor("fb1", [W, 2 * S], f32r, kind="ExternalInput").ap(),
        nc.dram_tensor("fb2", [W, 2 * S], f32r, kind="ExternalInput").ap(),
        nc.dram_tensor("cc1", [BINS, 256], f32r, kind="ExternalInput").ap(),
        nc.dram_tensor("cc2", [BINS, 256], f32r, kind="ExternalInput").ap(),
        nc.dram_tensor("wd1", [128, 256], f32r, kind="ExternalInput").ap(),
        nc.dram_tensor("wd2", [S - 128, 256], f32r, kind="ExternalInput").ap(),
    ]
    outs = [nc.dram_tensor("y", [n_ch, B, H, W], f32, kind="ExternalOutput").ap()]
    from contextlib import ExitStack
    with tile.TileContext(nc) as tc, ExitStack() as ctx:
        _build_core_kernel(ctx, nc, tc, ins, outs, n_ch)
    nc.compile()
    _COMPILED[n_ch] = nc
    return nc


def _prepare_in_maps(x, A_kernel, B_kernel, n_ch=CH_PER_CORE, n_cores=N_CORES):
    ehat = _host_effective_kernel(np.asarray(A_kernel), np.asarray(B_kernel))
    FA, FB1, FB2, CC1, CC2, WD1, WD2 = _host_consts()
    consts = {"fa": FA, "fb1": FB1, "fb2": FB2, "cc1": CC1, "cc2": CC2,
              "wd1": WD1, "wd2": WD2}
    # (B,H,W,C) -> (C,B,H,W) contiguous, shard channels
    xc = np.ascontiguousarray(np.moveaxis(np.asarray(x), 3, 0)).astype(np.float32)
    in_maps = []
    for s in range(n_cores):
        c0 = s * n_ch
        in_maps.append({
            "x": np.ascontiguousarray(xc[c0:c0 + n_ch]),
            "eh": np.ascontiguousarray(ehat[c0:c0 + n_ch]),
            **consts,
        })
    return in_maps


def _assemble(results, n_ch=CH_PER_CORE, n_cores=N_CORES):
    y = np.empty((B, H, W, C), np.float32)
    for s in range(n_cores):
        ys = results[s]["y"]                    # (n_ch, B, j, i) = Y^T planes
        y[:, :, :, s * n_ch:(s + 1) * n_ch] = np.transpose(ys, (1, 3, 2, 0))
    return y


def kernel(x, A_kernel, B_kernel):
    nc = _build(CH_PER_CORE)
    in_maps = _prepare_in_maps(x, A_kernel, B_kernel)
    res = bass_utils.run_bass_kernel_spmd(nc, in_maps, list(range(N_CORES)))
    return _assemble(res.results)
